# revision 1
# baseline (speedup 1.0000x reference)
"""TRN2 Bass kernel for nn_CrossAttention: B=8 data-parallel over 8 cores.

Per core (one batch element, T=2048 tokens):
  Q/K projections -> token-major SBUF; per-8-token-group block-diagonal
  matmul computes all 64x64 attention logit matrices on the PE at
  fp32r rate; exp on ACT; softmax denominator via segmented DVE reduce;
  second einsum as a grouped "garbage-diagonal" matmul in bf16; output
  regrouped feature-major through a DRAM + XBAR-transpose hop; final
  projection in bf16 on the PE.
"""
import sys
sys.path.insert(0, '/opt/trn_rl_repo')
import numpy as np
import ml_dtypes

import concourse.bass as bass
import concourse.bacc as bacc
import concourse.tile as tile
import concourse.mybir as mybir
from concourse.bass_utils import run_bass_kernel_spmd

f32r = mybir.dt.float32r
f32 = mybir.dt.float32
bf16 = mybir.dt.bfloat16
AX = mybir.AxisListType
AF = mybir.ActivationFunctionType

_CACHE = {}


def build(T=2048, C=256):
    assert T % C == 0 and C % 128 == 0
    TT = C // 128
    NCHUNK = T // C
    NG = C // 8  # 8-token groups per chunk

    nc = bacc.Bacc("TRN2", target_bir_lowering=False, debug=False)

    x1T = nc.dram_tensor("x1T", [1024, T], f32r, kind="ExternalInput").ap()
    x2T = nc.dram_tensor("x2T", [1024, T], f32r, kind="ExternalInput").ap()
    WqT = nc.dram_tensor("WqT", [1024, 1024], f32r, kind="ExternalInput").ap()
    WkT = nc.dram_tensor("WkT", [1024, 1024], f32r, kind="ExternalInput").ap()
    WvT = nc.dram_tensor("WvT", [1024, 1024], f32r, kind="ExternalInput").ap()
    WoT = nc.dram_tensor("WoT", [1024, 1024], bf16, kind="ExternalInput").ap()
    boR = nc.dram_tensor("boR", [128, 8, 256], f32, kind="ExternalInput").ap()
    yT = nc.dram_tensor("yT", [1024, T], f32, kind="ExternalOutput").ap()
    o2d = nc.dram_tensor("o2d", [T, 1024], bf16).ap()

    x1Tv = x1T.rearrange("(kf p) t -> p kf t", p=128)
    x2Tv = x2T.rearrange("(kf p) t -> p kf t", p=128)
    WqTv = WqT.rearrange("(kf p) f -> p kf f", p=128)
    WkTv = WkT.rearrange("(kf p) f -> p kf f", p=128)
    WvTv = WvT.rearrange("(kf p) f -> p kf f", p=128)
    WoTv = WoT.rearrange("(kf p) f -> p kf f", p=128)

    with tile.TileContext(nc) as tc:
        import contextlib
        ctx = contextlib.ExitStack()
        with ctx:
            P = {}
            P["w"] = ctx.enter_context(tc.tile_pool(name="w", bufs=1))
            P["xc"] = ctx.enter_context(tc.tile_pool(name="xc", bufs=1))
            P["qk"] = ctx.enter_context(tc.tile_pool(name="qk", bufs=1))
            P["kl"] = ctx.enter_context(tc.tile_pool(name="kl", bufs=4))
            P["E"] = ctx.enter_context(tc.tile_pool(name="E", bufs=6))
            P["sr"] = ctx.enter_context(tc.tile_pool(name="sr", bufs=6))
            P["vn"] = ctx.enter_context(tc.tile_pool(name="vn", bufs=1))
            P["ae"] = ctx.enter_context(tc.tile_pool(name="ae", bufs=3))
            P["o2"] = ctx.enter_context(tc.tile_pool(name="o2", bufs=1))
            P["ye"] = ctx.enter_context(tc.tile_pool(name="ye", bufs=2))
            P["ps"] = ctx.enter_context(
                tc.tile_pool(name="ps", bufs=8, space="PSUM"))

            Wq_s = P["w"].tile([128, 8, 1024], f32r)
            Wk_s = P["w"].tile([128, 8, 1024], f32r)
            Wv_s = P["w"].tile([128, 8, 1024], f32r)
            Wo_s = P["w"].tile([128, 8, 1024], bf16)
            nc.sync.dma_start(out=Wq_s, in_=WqTv)
            nc.sync.dma_start(out=Wk_s, in_=WkTv)
            nc.sync.dma_start(out=Wv_s, in_=WvTv)
            nc.sync.dma_start(out=Wo_s, in_=WoTv)
            boR_s = P["w"].tile([128, 8, 256], f32)
            nc.sync.dma_start(out=boR_s, in_=boR)

            # block-diag rhs buffers; zeros persist, diag blocks rewritten
            bd_bufs = []
            for i in range(4):
                t_ = nc.alloc_sbuf_tensor(f"bd{i}", [128, 512], f32r)
                nc.vector.memset(t_.ap().bitcast(f32), 0.0)
                bd_bufs.append(t_)

            for ci in range(NCHUNK):
                c0 = ci * C
                x1c = P["xc"].tile([128, 8, C], f32r, tag="x1c")
                x2c = P["xc"].tile([128, 8, C], f32r, tag="x2c")
                nc.sync.dma_start(out=x1c, in_=x1Tv[:, :, c0:c0 + C])
                nc.sync.dma_start(out=x2c, in_=x2Tv[:, :, c0:c0 + C])

                Qc = P["qk"].tile([128, TT, 1024], f32r, tag="Qc")
                Kc = P["qk"].tile([128, TT, 1024], f32r, tag="Kc")
                for dst, W_s, xc in ((Qc, Wq_s, x1c), (Kc, Wk_s, x2c)):
                    for tt in range(TT):
                        for fh in range(2):
                            ps = P["ps"].tile([128, 512], f32, tag="ps")
                            for kf in range(8):
                                nc.tensor.matmul(
                                    ps, xc[:, kf, tt * 128:(tt + 1) * 128],
                                    W_s[:, kf, fh * 512:(fh + 1) * 512],
                                    start=(kf == 0), stop=(kf == 7))
                            nc.scalar.activation(
                                dst[:, tt, fh * 512:(fh + 1) * 512], ps, AF.Copy)

                # V projection, h-split -> v2T [64v, (t,h)] bf16
                v2T = P["vn"].tile([64, C * 16], bf16, tag="vn")
                v2Tv = v2T.rearrange("p (t h) -> p t h", h=16)
                for h in range(16):
                    ps_v = P["ps"].tile([64, C], f32, tag="ps")
                    for kf in range(8):
                        nc.tensor.matmul(
                            ps_v, Wv_s[:, kf, h * 64:(h + 1) * 64],
                            x2c[:, kf, :], start=(kf == 0), stop=(kf == 7))
                    nc.vector.tensor_copy(v2Tv[:, :, h], ps_v)

                WQ = TT * 1024
                for g in range(NG):
                    tau0 = g * 8  # in-chunk first token of group
                    tt = tau0 // 128
                    p0 = tau0 % 128
                    klhsT = P["kl"].tile([128, 64], f32r, tag="kl")
                    bd = bd_bufs[g % 4]
                    for t in range(8):
                        src = bass.AP(
                            tensor=Kc.tensor,
                            offset=Kc.offset + (p0 + t) * WQ + tt * 1024,
                            ap=[[WQ, 1], [64, 16], [1, 64]])
                        dst = bass.AP(
                            tensor=klhsT.tensor,
                            offset=klhsT.offset + t * 16 * 64,
                            ap=[[64, 16], [1, 64]])
                        nc.sync.dma_start(out=dst, in_=src)
                        srcq = bass.AP(
                            tensor=Qc.tensor,
                            offset=Qc.offset + (p0 + t) * WQ + tt * 1024,
                            ap=[[WQ, 1], [64, 16], [1, 64]])
                        dstq = bass.AP(
                            tensor=bd,
                            offset=t * 16 * 512 + t * 64,
                            ap=[[512, 16], [1, 64]])
                        nc.sync.dma_start(out=dstq, in_=srcq)

                    ps_b = P["ps"].tile([64, 512], f32, tag="ps")
                    nc.tensor.matmul(ps_b, klhsT, bd.ap(),
                                     start=True, stop=True)
                    E = P["E"].tile([64, 512], bf16, tag="E")
                    nc.scalar.activation(E, ps_b, AF.Exp, scale=0.125)
                    Ev = E.rearrange("p (t d) -> p t d", d=64)
                    S = P["sr"].tile([64, 8], f32, tag="S")
                    nc.vector.reduce_sum(S, Ev, axis=AX.X)
                    R = P["sr"].tile([64, 8], f32, tag="R")
                    nc.vector.reciprocal(R, S)
                    nc.vector.tensor_mul(
                        Ev, Ev, R.unsqueeze(2).to_broadcast([64, 8, 64]))

                    # alpha: one garbage-diagonal matmul per group
                    ps_a = P["ps"].tile([128, 512], f32, tag="ps")
                    nc.tensor.matmul(
                        ps_a, v2T[:, tau0 * 16:(tau0 + 8) * 16], E,
                        start=True, stop=True)
                    aev = P["ae"].tile([128, 512], bf16, tag="ae")
                    if g % 2 == 0:
                        nc.vector.tensor_copy(aev, ps_a)
                    else:
                        nc.scalar.activation(aev, ps_a, AF.Copy)
                    # valid diag blocks -> DRAM out2 token-major bf16
                    for t in range(8):
                        src = bass.AP(
                            tensor=aev.tensor,
                            offset=aev.offset + (t * 16) * 512 + t * 64,
                            ap=[[512, 16], [1, 64]])
                        dst = bass.AP(
                            tensor=o2d.tensor,
                            offset=(c0 + tau0 + t) * 1024,
                            ap=[[64, 16], [1, 64]])
                        nc.sync.dma_start(out=dst, in_=src)

                # out2T via XBAR transpose: [C,128] -> [128,C] per kf
                out2T = P["o2"].tile([128, 8, C], bf16, tag="o2")
                for kf in range(8):
                    nc.sync.dma_start(
                        out=out2T[:, kf, :],
                        in_=o2d[c0:c0 + C, kf * 128:(kf + 1) * 128],
                        transpose=True)

                for st in range(8):
                    ps_y = P["ps"].tile([128, C], f32, tag="ps")
                    for kf in range(8):
                        nc.tensor.matmul(
                            ps_y, Wo_s[:, kf, st * 128:(st + 1) * 128],
                            out2T[:, kf, :], start=(kf == 0), stop=(kf == 7))
                    yTs = P["ye"].tile([128, C], f32, tag="ye")
                    nc.vector.tensor_add(yTs, ps_y, boR_s[:, st, 0:C])
                    nc.sync.dma_start(
                        out=yT[st * 128:(st + 1) * 128, c0:c0 + C], in_=yTs)

    nc.compile()
    return nc


def kernel(x1, x2, Wq, Wk, Wv, Wo, bo):
    x1 = np.asarray(x1, dtype=np.float32)
    x2 = np.asarray(x2, dtype=np.float32)
    Wq = np.asarray(Wq, dtype=np.float32)
    Wk = np.asarray(Wk, dtype=np.float32)
    Wv = np.asarray(Wv, dtype=np.float32)
    Wo = np.asarray(Wo, dtype=np.float32)
    bo = np.asarray(bo, dtype=np.float32)
    B, M, _ = x1.shape
    if "nc" not in _CACHE:
        _CACHE["nc"] = build(T=M, C=256)
    nc = _CACHE["nc"]

    shared = {
        "WqT": np.ascontiguousarray(Wq.T),
        "WkT": np.ascontiguousarray(Wk.T),
        "WvT": np.ascontiguousarray(Wv.T),
        "WoT": np.ascontiguousarray(Wo.T).astype(ml_dtypes.bfloat16),
        "boR": np.ascontiguousarray(
            np.broadcast_to(bo.reshape(8, 128).T[:, :, None], (128, 8, 256))),
    }
    in_maps = []
    for b in range(B):
        im = dict(shared)
        im["x1T"] = np.ascontiguousarray(x1[b].T)
        im["x2T"] = np.ascontiguousarray(x2[b].T)
        in_maps.append(im)
    res = run_bass_kernel_spmd(nc, in_maps, core_ids=list(range(8)))
    out = np.stack([res.results[b]["yT"].T for b in range(B)], axis=0)
    return out.astype(np.float32)
